# revision 10
# baseline (speedup 1.0000x reference)
"""Trainium2 Bass kernel for nn_DelayExpansionLayer (histogram_binning).

Computation: per-channel mean of layer_output [64,256,56,56] over (B,H,W),
round to 1e-6, nearest-key lookup in a sorted 1024-entry table, max over
channels, scale by (in_ch*out_ch)/512, broadcast to (56,56).

Strategy (data-parallel over batch, 8 NeuronCores):
  - Each core gets 8 batches = [8,256,56,56] (25.7 MB) and computes
    per-channel partial sums on-device (DMA-line-rate bound: 16 SDMA
    engines x ~26.5 GB/s = ~61 us stream).
  - Host combines the 8 partial-sum vectors, then does the O(C+K)
    lookup/max/broadcast epilogue.

Per-core device kernel (raw bass, manual semaphores):
  input  x [8, 128, 2, 3136] f32  (batch, partition, chan-pair, spatial).
  8 SBUF slots (one per batch, ~201KB/partition) and a dedicated
  semaphore per input DMA -> every DMA is issued up front with no
  issue-gating chain, so a straggling SDMA engine can never idle the
  other 15 (v2 lesson). Batches 0-5 stream as full 3.2MB DMAs reduced
  whole (DVE: b0/b2/b4 via tensor_reduce; ACT: b1/b3/b5 via
  activation-Copy accum). Batches 6-7 are tapered into 6 chunks
  (1792/1344 | 1136/768/672/560 pair-cols, from an offline 2-engine
  schedule sim with fitted op costs: DVE 615ns + el/1.03, ACT 480ns +
  el/1.31 + 330ns accumulator readout) mostly X-split j0->DVE /
  j1->ACT so both engines drain the tail together and finish ~1.2us
  after the last byte. Partial sums land in a flat stats[128,24]
  (cell c -> cols 2c/2c+1 for j0/j1) so each output DMA is one
  contiguous run per partition. The two output DMAs are issued from
  the Scalar engine (ACT's own HWDGE ring, row 10) so they neither
  queue behind the input ring nor suffer its descriptor backpressure.
"""

import sys
import types

import numpy as np

N_CORES = 8
B_FULL, C, H, W = 64, 256, 56, 56
HW = H * W
B_LOCAL = B_FULL // N_CORES
SCALE_DENOM = 32 * 16

# Set by a test harness to enable NTFF tracing of the SPMD run.
TRACE = False
TRACE_TMPDIR = None
LAST_RESULTS = None

_CACHE = {}

# Tail chunks: (batch, pair-col start, end, type); V = pair on DVE,
# X = j0 on DVE + j1 on ACT. Cell index = 6 + position. The last chunk
# is V-only so the final out DMA depends on no ACT-accumulator
# writeback (DVE writes stats directly; vd fires post-write).
TAIL_CHUNKS = (
    (6, 0, 1792, "X"),
    (6, 1792, 3136, "V"),
    (7, 0, 1280, "X"),
    (7, 1280, 2080, "X"),
    (7, 2080, 2656, "X"),
    (7, 2656, 3136, "V"),
)
N_CELLS = 6 + len(TAIL_CHUNKS)
STATS_COLS = 2 * N_CELLS
OUT1_VD = 8  # V ops covered by out1 (all but the final V chunk)
OUT1_AD = 7  # all ACT ops: batches 1,3,5 + 4 X-chunk j1's
OUT2_VD = 9  # all V ops


def _ensure_axon_hooks_shim():
    """bass_utils' axon trace path imports antenv.axon_hooks; provide a
    no-op shim when the environment's antenv package lacks it."""
    try:
        import antenv.axon_hooks  # noqa: F401
        return
    except ImportError:
        pass

    mod = types.ModuleType("antenv.axon_hooks")
    _hook = [None]
    mod.set_axon_ntff_profile_hook = lambda h: _hook.__setitem__(0, h)
    mod.get_axon_ntff_profile_hook = lambda: _hook[0]
    sys.modules["antenv.axon_hooks"] = mod
    try:
        import antenv

        antenv.axon_hooks = mod
    except ImportError:
        pass


def _build():
    if "nc" in _CACHE:
        return _CACHE["nc"]
    import concourse.bass as bass
    from concourse import mybir

    nc = bass.Bass(
        "TRN2",
        target_bir_lowering=False,
        debug=False,
        enable_asserts=False,
        num_devices=N_CORES,
    )
    f32 = mybir.dt.float32
    x = nc.dram_tensor("x", [B_LOCAL, 128, 2, HW], f32, kind="ExternalInput").ap()
    out = nc.dram_tensor("out", [128, STATS_COLS], f32, kind="ExternalOutput").ap()

    slots = [
        nc.alloc_sbuf_tensor(f"slot{i}", [128, 2, HW], f32).ap() for i in range(8)
    ]
    stats = nc.alloc_sbuf_tensor("stats", [128, STATS_COLS], f32).ap()

    with (
        nc.Block(no_gpsimd_drain=True) as block,
        nc.semaphore("s0") as s0,
        nc.semaphore("s1") as s1,
        nc.semaphore("s2") as s2,
        nc.semaphore("s3") as s3,
        nc.semaphore("s4") as s4,
        nc.semaphore("s5") as s5,
        nc.semaphore("c0") as c0,
        nc.semaphore("c1") as c1,
        nc.semaphore("c2") as c2,
        nc.semaphore("c3") as c3,
        nc.semaphore("c4") as c4,
        nc.semaphore("c5") as c5,
        nc.semaphore("vd") as vd,
        nc.semaphore("ad") as ad,
        nc.semaphore("od") as od,
    ):
        bsems = [s0, s1, s2, s3, s4, s5]
        csems = [c0, c1, c2, c3, c4, c5]

        @block.sync
        def _(sync: bass.BassEngine):
            # every input DMA up front, dedicated sems, no gating
            for b in range(6):
                sync.dma_start(out=slots[b][:], in_=x[b]).then_inc(bsems[b], 16)
            for k, (b, p0, p1, _typ) in enumerate(TAIL_CHUNKS):
                sync.dma_start(
                    out=slots[b][:, :, p0:p1], in_=x[b][:, :, p0:p1]
                ).then_inc(csems[k], 16)
            # final out: the last (V-only) cell — issued here so it
            # overlaps Scalar's out1 issue instead of queueing behind it
            sync.wait_ge(vd, OUT2_VD)
            sync.dma_start(
                out=out[:, STATS_COLS - 2 : STATS_COLS],
                in_=stats[:, STATS_COLS - 2 : STATS_COLS],
            ).then_inc(od, 16)
            sync.wait_ge(od, 32)

        @block.vector
        def _(vector: bass.BassEngine):
            # full batches 0,2,4 -> cells 0,2,4 (flat cols 2b..2b+1)
            for b in (0, 2, 4):
                vector.wait_ge(bsems[b], 16)
                vector.reduce_sum(
                    stats[:, 2 * b : 2 * b + 2],
                    slots[b][:],
                    axis=mybir.AxisListType.X,
                ).then_inc(vd, 1)
            # tail chunks: V side
            for k, (b, p0, p1, typ) in enumerate(TAIL_CHUNKS):
                vector.wait_ge(csems[k], 16)
                col = 2 * (6 + k)
                if typ == "V":
                    vector.reduce_sum(
                        stats[:, col : col + 2],
                        slots[b][:, :, p0:p1],
                        axis=mybir.AxisListType.X,
                    ).then_inc(vd, 1)
                else:  # X: j0 on DVE
                    vector.reduce_sum(
                        stats[:, col : col + 1],
                        slots[b][:, 0, p0:p1],
                        axis=mybir.AxisListType.X,
                    ).then_inc(vd, 1)

        @block.scalar
        def _(scalar: bass.BassEngine):
            # full batches 1,3,5 -> cells 1,3,5; the ad inc on the j1 ACT
            # fires after the accumulator writeback, ordering stats
            # visibility for the out DMAs.
            for b in (1, 3, 5):
                scalar.wait_ge(bsems[b], 16)
                for j in range(2):
                    ins = scalar.activation(
                        slots[b][:, j, :],
                        slots[b][:, j, :],
                        mybir.ActivationFunctionType.Copy,
                        accum_out=stats[:, 2 * b + j : 2 * b + j + 1],
                    )
                    if j == 1:
                        ins.then_inc(ad, 1)
            # tail chunks: ACT side (j1 of X chunks)
            for k, (b, p0, p1, typ) in enumerate(TAIL_CHUNKS):
                if typ != "X":
                    continue
                scalar.wait_ge(csems[k], 16)
                scalar.activation(
                    slots[b][:, 1, p0:p1],
                    slots[b][:, 1, p0:p1],
                    mybir.ActivationFunctionType.Copy,
                    accum_out=stats[:, 2 * (6 + k) + 1 : 2 * (6 + k) + 2],
                ).then_inc(ad, 1)
            # early out: cols 0:22 — every ACT-written cell plus V cells
            # 0..7 (ad wait flushes scalar's own accumulator writebacks)
            scalar.wait_ge(ad, OUT1_AD)
            scalar.wait_ge(vd, OUT1_VD)
            scalar.dma_start(
                out=out[:, 0 : STATS_COLS - 2], in_=stats[:, 0 : STATS_COLS - 2]
            ).then_inc(od, 16)

    _CACHE["nc"] = nc
    return nc


def kernel(layer_output, delay_keys, delay_values, in_channels, out_channels):
    global LAST_RESULTS
    _ensure_axon_hooks_shim()
    from concourse.bass_utils import run_bass_kernel_spmd

    x = np.ascontiguousarray(np.asarray(layer_output, dtype=np.float32))
    assert x.shape == (B_FULL, C, H, W), x.shape
    # shard over batch; view channels as (partition, pair): c = 2*p + j
    xr = x.reshape(N_CORES, B_LOCAL, 128, 2, HW)
    in_maps = [{"x": xr[k]} for k in range(N_CORES)]

    nc = _build()
    kwargs = {}
    if TRACE:
        kwargs.update(trace=True, tmpdir=TRACE_TMPDIR)
    res = run_bass_kernel_spmd(nc, in_maps, core_ids=list(range(N_CORES)), **kwargs)
    LAST_RESULTS = res

    # tiny [C] all-reduce of the per-core partial sums
    parts = np.stack(
        [res.results[k]["out"] for k in range(N_CORES)]
    )  # [8, 128, 24]; cell c -> cols (2c: j0, 2c+1: j1), all cells valid
    s0 = parts[:, :, 0::2].sum(axis=(0, 2), dtype=np.float32)  # j=0
    s1 = parts[:, :, 1::2].sum(axis=(0, 2), dtype=np.float32)  # j=1
    sums = np.stack([s0, s1], axis=1).reshape(C)  # c = 2p+j
    means = sums / np.float32(B_FULL * HW)
    means = np.round(means * np.float32(1e6)) / np.float32(1e6)

    keys = np.asarray(delay_keys, dtype=np.float32)
    values = np.asarray(delay_values, dtype=np.float32)
    K = keys.shape[0]
    idx = np.searchsorted(keys, means)
    lo = np.clip(idx - 1, 0, K - 1)
    hi = np.clip(idx, 0, K - 1)
    pick_hi = np.abs(keys[hi] - means) < np.abs(keys[lo] - means)
    nearest = np.where(pick_hi, hi, lo)
    merged = np.float32(values[nearest].max())

    scale = np.float32(
        (int(np.asarray(in_channels)) * int(np.asarray(out_channels))) / SCALE_DENOM
    )
    return np.full((H, W), merged, dtype=np.float32) * scale
